# revision 67
# baseline (speedup 1.0000x reference)
"""Trainium2 Bass kernel for nn_BertAttentionEx (BERT attention with
relative_key_query position embeddings + output dense + residual + LayerNorm).

Distribution: 8 cores = 4 batches x 2 head-groups (8 heads each).

v2 design (vs v1): no xbar DMA transposes. Both relative-position terms are
computed as dense "band" matmuls against the (reversed) distance table,
stored fp8 partition-major in DRAM, and read back with a single skewed
strided DMA per (head, side):
  - k-side term arrives directly in transposed-score layout [r', l] and is
    added on DVE during PSUM evacuation;
  - q-side term arrives in natural layout [l', r] and is transposed into the
    score PSUM by plain PE matmuls against an fp8 identity (accumulating).
Transposed softmax (scores kept as s^T), v augmented with a ones column so
softmax normalizers fall out of the PV matmul, partial output dense, pairwise
ReduceScatter (bf16), then residual + LayerNorm on each core's row-half.
"""
import sys
import numpy as np
import ml_dtypes
from contextlib import ExitStack

sys.path.insert(0, "/opt/trn_rl_repo")

import concourse.bass as bass
import concourse.bacc as bacc
import concourse.tile as tile
from concourse import mybir
from concourse.bass_utils import run_bass_kernel_spmd

B, S, HID = 4, 1024, 1024
NH, HD = 16, 64
MAX_POS = 1024
LN_EPS = 1e-12
NCORES = 8
HPC = 8           # heads per core
W = 1152          # band width per 128-row tile
BT = S // 128     # 8 row tiles
F32 = mybir.dt.float32
F32R = mybir.dt.float32r
BF16 = mybir.dt.bfloat16
FP8 = mybir.dt.float8e4
AF = mybir.ActivationFunctionType
ALU = mybir.AluOpType

_COMPILED = None


def r32(ap):
    return ap.bitcast(F32R)


def build_program():
    nc = bacc.Bacc("TRN2", target_bir_lowering=False, debug=False,
                   num_devices=NCORES)

    # ---- per-core external I/O (host pre-casts to bf16 where noted) ----
    hsT = nc.declare_dram_parameter("hsT", [HID, S], BF16, isOutput=False)
    res = nc.declare_dram_parameter("res", [S // 2, HID], BF16, isOutput=False)
    wqT = nc.declare_dram_parameter("wqT", [HID, 512], BF16, isOutput=False)
    wkT = nc.declare_dram_parameter("wkT", [HID, 512], BF16, isOutput=False)
    wvT = nc.declare_dram_parameter("wvT", [HID, 520], BF16, isOutput=False)
    bqv = nc.declare_dram_parameter("bq", [128, 4], F32, isOutput=False)
    bkv = nc.declare_dram_parameter("bk", [128, 4], F32, isOutput=False)
    bvaug = nc.declare_dram_parameter("bvaug", [520], F32, isOutput=False)
    drTt = nc.declare_dram_parameter("drT", [128, 2048], BF16, isOutput=False)
    dTt = nc.declare_dram_parameter("dT", [128, 2048], BF16, isOutput=False)
    woT = nc.declare_dram_parameter("woT", [512, HID], BF16, isOutput=False)
    maskc = nc.declare_dram_parameter("maskc", [128, 8], F32, isOutput=False)
    ident8 = nc.declare_dram_parameter("ident8", [128, 128], FP8, isOutput=False)
    ones64 = nc.declare_dram_parameter("ones64", [128, 64], BF16, isOutput=False)
    lng = nc.declare_dram_parameter("lng", [HID], F32, isOutput=False)
    lnb = nc.declare_dram_parameter("lnb", [HID], F32, isOutput=False)
    out = nc.declare_dram_parameter("out", [S // 2, HID], F32, isOutput=True)

    # internal DRAM: fp8 bands, partition-major [p, t, j] (pitch 8*W per p)
    bandA = [nc.dram_tensor(f"bandA{i}", [128, BT, W], FP8) for i in range(8)]
    bandB = [nc.dram_tensor(f"bandB{i}", [128, BT, W], FP8) for i in range(8)]
    attn_part = nc.dram_tensor("attn_part", [S, HID], BF16)
    rs_out = nc.dram_tensor("rs_out", [S // 2, HID], BF16)

    PIT = BT * W  # per-partition band pitch (elements)

    with ExitStack() as ctx:
        tc = ctx.enter_context(tile.TileContext(nc))
        consts = ctx.enter_context(tc.tile_pool(name="consts", bufs=1))
        persist = ctx.enter_context(tc.tile_pool(name="persist", bufs=1))
        wpool = ctx.enter_context(tc.tile_pool(name="wpool", bufs=2))
        hsp = ctx.enter_context(tc.tile_pool(name="hsp", bufs=8))
        bigp = ctx.enter_context(tc.tile_pool(name="bigp", bufs=5))
        # one pool for band staging AND the B2 gathers: the band tiles are
        # dead once read back, so the slots recycle into gather buffers
        bandsb = ctx.enter_context(tc.tile_pool(name="bandsb", bufs=7))
        ppool = ctx.enter_context(tc.tile_pool(name="ppool", bufs=3))
        misc = ctx.enter_context(tc.tile_pool(name="misc", bufs=2))
        ctxup = ctx.enter_context(tc.tile_pool(name="ctxup", bufs=8))
        lns = ctx.enter_context(tc.tile_pool(name="lns", bufs=2))
        psP = ctx.enter_context(tc.tile_pool(name="psP", bufs=4, space="PSUM"))
        psCtx = ctx.enter_context(tc.tile_pool(name="psCtx", bufs=2, space="PSUM"))

        # ---- constants ----
        drT_sb = consts.tile([128, 2048], BF16)
        nc.sync.dma_start(out=drT_sb, in_=drTt[:, :])
        dT_sb = consts.tile([128, 2048], BF16)
        nc.sync.dma_start(out=dT_sb, in_=dTt[:, :])
        bq_sb = consts.tile([128, 4], F32)
        nc.sync.dma_start(out=bq_sb, in_=bqv[:, :])
        bk_sb = consts.tile([128, 4], F32)
        nc.sync.dma_start(out=bk_sb, in_=bkv[:, :])
        bv_bc = consts.tile([128, 520], F32)
        nc.sync.dma_start(
            out=bv_bc,
            in_=bass.AP(tensor=bvaug, offset=0, ap=[[0, 128], [1, 520]]),
        )
        mask_sb = consts.tile([128, 8], F32)
        nc.sync.dma_start(out=mask_sb, in_=maskc[:, :])
        id8_sb = consts.tile([128, 128], FP8)
        nc.sync.dma_start(out=id8_sb, in_=ident8[:, :])
        lng_bc = consts.tile([128, HID], BF16)
        nc.gpsimd.dma_start(
            out=lng_bc,
            in_=bass.AP(tensor=lng, offset=0, ap=[[0, 128], [1, HID]]),
        )
        lnb_bc = consts.tile([128, HID], BF16)
        nc.gpsimd.dma_start(
            out=lnb_bc,
            in_=bass.AP(tensor=lnb, offset=0, ap=[[0, 128], [1, HID]]),
        )
        eps_sb = consts.tile([128, 1], F32)
        nc.vector.memset(eps_sb, LN_EPS)
        ones_row = consts.tile([128, 64], BF16)
        nc.sync.dma_start(out=ones_row, in_=ones64[:, :])

        # ---- persistent activations ----
        qT_sb = persist.tile([128, 4, S], BF16, tag="qT")    # [d, l], 2 heads/tile
        kT_sb = persist.tile([128, 4, S], BF16, tag="kT")
        vv_sb = persist.tile([128, 8, 520], BF16, tag="vv")  # v natural [r, 65h+..]
        ctxP = [persist.tile([64, 2, S], BF16, tag=f"ctxP{i}", name=f"ctxP{i}")
                for i in range(4)]                           # head h -> tile h//2, slot h%2

        # ---- phase A: load hs^T, projections (q, k serialized, then v) ----
        hsT_tiles = []
        for kc in range(8):
            htile = hsp.tile([128, S], BF16, tag="hst", name=f"hsT{kc}")
            nc.sync.dma_start(out=htile, in_=hsT[128 * kc:128 * kc + 128, :])
            hsT_tiles.append(htile)

        for (wsrc, b_sb, dst) in ((wqT, bq_sb, qT_sb), (wkT, bk_sb, kT_sb)):
            w_sb = wpool.tile([128, 8, 520], BF16, tag="w", name=f"w_{dst.name}")
            for kc in range(8):
                nc.sync.dma_start(out=w_sb[:, kc, 0:512],
                                  in_=wsrc[128 * kc:128 * kc + 128, :])
            for i in range(4):
                for nh2 in range(2):
                    ps = psP.tile([128, 512], F32, tag="ps", name=f"ps_{dst.name}_{i}_{nh2}")
                    for kc in range(8):
                        nc.tensor.matmul(
                            ps,
                            lhsT=w_sb[:, kc, 128 * i:128 * i + 128],
                            rhs=hsT_tiles[kc][:, 512 * nh2:512 * nh2 + 512],
                            start=(kc == 0), stop=(kc == 7),
                        )
                    nc.scalar.activation(
                        out=dst[:, i, 512 * nh2:512 * nh2 + 512],
                        in_=ps, func=AF.Identity,
                        bias=b_sb[:, i:i + 1], scale=1.0,
                    )
        wv_sb = wpool.tile([128, 8, 520], BF16, tag="w")
        for kc in range(8):
            nc.sync.dma_start(out=wv_sb[:, kc, :],
                              in_=wvT[128 * kc:128 * kc + 128, :])

        def emit_v_block(rt):
            # one row-tile of the v projection; interleaved into the band
            # phase where the PE is otherwise evac-starved
            for (c0, cn) in ((0, 260), (260, 260)):
                ps = psP.tile([128, 512], F32, tag="ps", name=f"ps_v_{rt}_{c0}")
                for kc in range(8):
                    nc.tensor.matmul(
                        ps[:, 0:cn],
                        lhsT=hsT_tiles[kc][:, 128 * rt:128 * rt + 128],
                        rhs=wv_sb[:, kc, c0:c0 + cn],
                        start=(kc == 0), stop=(kc == 7),
                    )
                nc.vector.tensor_tensor(
                    out=vv_sb[:, rt, c0:c0 + cn],
                    in0=ps[:, 0:cn], in1=bv_bc[:, c0:c0 + cn], op=ALU.add,
                )

        for rt in range(8):
            emit_v_block(rt)

        # ---- phase B1: band matmuls -> fp8 SBUF -> one DMA per (head, side) ----
        # Emitted per head pair with even/odd heads on PE row groups 0-63 /
        # 64-127 so consecutive K=64 matmuls overlap in the array.
        def issue_gathers(h):
            # Skewed strided reads from the partition-major fp8 bands:
            #   gk[r', u, l]   = bandB[h][r', u, 127 - r' + l]  (score^T layout)
            #   aex[l', t, r]  = bandA[h][l', t, 127 - l' + r]  (natural layout)
            gk = bandsb.tile([128, BT, S], FP8, tag="bandsb", name=f"gk{h}")
            nc.sync.dma_start(
                out=gk,
                in_=bass.AP(tensor=bandB[h], offset=127,
                            ap=[[PIT - 1, 128], [W, BT], [1, S]]),
            )
            aex = bandsb.tile([128, BT, S], FP8, tag="bandsb", name=f"aex{h}")
            nc.sync.dma_start(
                out=aex,
                in_=bass.AP(tensor=bandA[h], offset=127,
                            ap=[[PIT - 1, 128], [W, BT], [1, S]]),
            )
            return gk, aex

        gather0 = None
        for hp in range(HPC // 2):
            for (bsrc, table, bufs_, nm) in ((qT_sb, drT_sb, bandA, "A"),
                                             (kT_sb, dT_sb, bandB, "Bb")):
                bsb = [bandsb.tile([128, BT, W], FP8, tag="bandsb",
                                   name=f"bsb{nm}{hp}_{hh}") for hh in range(2)]
                for t in range(BT):
                    j0 = 896 - 128 * t
                    # chunk-level interleave: consecutive matmuls target
                    # opposite PE row groups so they execute concurrently
                    for (c0, cn) in ((0, 512), (512, 512), (1024, 128)):
                        for hh in range(2):
                            hb = 64 * hh
                            ps = psP.tile([128, 512], F32, tag="ps",
                                          name=f"psb{nm}{hp}{hh}_{t}_{c0}")
                            nc.tensor.matmul(
                                ps[:, 0:cn],
                                lhsT=bsrc[hb:hb + 64, hp, 128 * t:128 * t + 128],
                                rhs=table[hb:hb + 64, j0 + c0:j0 + c0 + cn],
                                start=True, stop=True,
                            )
                            # alternate evac engine so neither ACT nor DVE
                            # rate-limits the PE during the band phase
                            if hh == 0:
                                nc.scalar.copy(out=bsb[hh][:, t, c0:c0 + cn],
                                               in_=ps[:, 0:cn])
                            else:
                                nc.vector.tensor_copy(out=bsb[hh][:, t, c0:c0 + cn],
                                                      in_=ps[:, 0:cn])
                for hh in range(2):
                    nc.sync.dma_start(
                        out=bass.AP(tensor=bufs_[2 * hp + hh], offset=0,
                                    ap=[[PIT, 128], [1, PIT]]),
                        in_=bsb[hh],
                    )
            if hp == 0:
                # prefetch head 0's gathers while pairs 1-3 compute bands,
                # hiding the band->B2 transition latency
                gather0 = issue_gathers(0)

        # ---- phase B2: attention per head pair (row-group concurrency) ----
        def emit_head_tail(h, ctxU, lh, on_act=False):
            # z-normalization + ctx scale for (head h, half lh), working off
            # the SBUF copy of the raw accumulator (PSUM long released):
            # Z sits on the ones-row (partition 64) of ctxU; broadcast it to
            # 64 partitions with a K=1 ones-matmul, reciprocal, scale.
            zps = psP.tile([64, 512], F32, tag="ps", name=f"zps{h}_{lh}")
            nc.tensor.matmul(
                zps,
                lhsT=ones_row[64:65, :],
                rhs=ctxU[lh][64:65, :],
                start=True, stop=True,
            )
            zrec = misc.tile([64, 512], F32, tag="zrec", name=f"zrec{h}_{lh}")
            if on_act:
                # final flush: the slow DVE reciprocal would sit on the
                # serial tail; compute 1/z as exp(-ln(z)) on the idle ACT
                # engine instead (z > 0 always)
                nc.scalar.activation(out=zps, in_=zps, func=AF.Ln)
                nc.scalar.activation(out=zrec, in_=zps, func=AF.Exp, scale=-1.0)
            else:
                nc.vector.reciprocal(out=zrec, in_=zps)
            nc.vector.tensor_tensor(
                out=ctxP[h // 2][:, h % 2, 512 * lh:512 * lh + 512],
                in0=ctxU[lh][0:64, :], in1=zrec, op=ALU.mult,
            )

        gathers = [gather0, issue_gathers(1)]
        pending_tails = []
        for hp2 in range(HPC // 2):
            h0 = 2 * hp2
            gk0, aex0 = gathers[0]
            gk1, aex1 = gathers[1]
            if h0 + 2 < HPC:
                gathers = [issue_gathers(h0 + 2), issue_gathers(h0 + 3)]

            cps = [[psCtx.tile([65, 512], F32, tag=f"ctx{lh}",
                               name=f"cps{h0 + hh}_{lh}") for lh in range(2)]
                   for hh in range(2)]
            pts = [None, None]
            for u in range(BT):
                if u == 1 and pending_tails:
                    for tail in pending_tails:
                        emit_head_tail(*tail)
                    pending_tails = []
                for hh in range(2):
                    pts[hh] = ppool.tile([128, S], BF16, tag="pt",
                                         name=f"pt{h0 + hh}_{u}")
                for lh in range(2):
                    sps = [None, None]
                    # both heads' QK matmuls back to back: K=64 each on
                    # opposite PE row groups -> they execute concurrently
                    for hh in range(2):
                        hb = 64 * hh
                        sp = psP.tile([128, 512], F32, tag="ps",
                                      name=f"sp{h0 + hh}_{u}_{lh}")
                        sps[hh] = sp
                        nc.tensor.matmul(
                            sp,
                            lhsT=kT_sb[hb:hb + 64, hp2, 128 * u:128 * u + 128],
                            rhs=qT_sb[hb:hb + 64, hp2, 512 * lh:512 * lh + 512],
                            start=True, stop=False,
                        )
                    for hh, aex_sb in ((0, aex0), (1, aex1)):
                        sp = sps[hh]
                        # q-side relative term: transpose natural-layout aex
                        # into the score PSUM via identity-rhs matmuls.
                        for i in range(4):
                            t = 4 * lh + i
                            nc.tensor.matmul(
                                sp[:, 128 * i:128 * i + 128],
                                lhsT=aex_sb[:, t, 128 * u:128 * u + 128],
                                rhs=id8_sb,
                                start=False, stop=False,
                            )
                    for hh, gk_sb in ((0, gk0), (1, gk1)):
                        sp = sps[hh]
                        # k-side relative term: injected by the PE as well
                        # (identity stationary, gk streams) so no vector op
                        # ever touches the score PSUM; exp reads PSUM direct.
                        nc.tensor.matmul(
                            sp,
                            lhsT=id8_sb,
                            rhs=gk_sb[:, u, 512 * lh:512 * lh + 512],
                            start=False, stop=True,
                        )
                        nc.scalar.activation(
                            out=pts[hh][:, 512 * lh:512 * lh + 512], in_=sp,
                            func=AF.Exp, bias=mask_sb[:, u:u + 1], scale=0.125,
                        )
                for hh in range(2):
                    for lh in range(2):
                        nc.tensor.matmul(
                            cps[hh][lh],
                            lhsT=vv_sb[:, u, 65 * (h0 + hh):65 * (h0 + hh) + 65],
                            rhs=pts[hh][:, 512 * lh:512 * lh + 512],
                            start=(u == 0), stop=(u == 7),
                        )
            # evacuate the raw accumulators to SBUF right away: releases the
            # psCtx banks for the next pair and takes the z-path off the PE
            # critical path entirely
            ctxUs = []
            for hh in range(2):
                cu = [ctxup.tile([128, 512], BF16, tag="ctxu",
                                 name=f"ctxU{h0 + hh}_{lh}") for lh in range(2)]
                for lh in range(2):
                    nc.vector.tensor_copy(out=cu[lh][0:65, :], in_=cps[hh][lh])
                ctxUs.append(cu)
            pending_tails = [(h0, ctxUs[0], 0), (h0, ctxUs[0], 1),
                             (h0 + 1, ctxUs[1], 0), (h0 + 1, ctxUs[1], 1)]
        for tail in pending_tails:
            emit_head_tail(*tail, on_act=True)
        pending_tails = []

        # ---- phase C: output dense (partial), ReduceScatter, LayerNorm ----
        wo_sb = wpool.tile([128, 4, HID], BF16, tag="w")
        for kc in range(4):
            nc.sync.dma_start(out=wo_sb[:, kc, :], in_=woT[128 * kc:128 * kc + 128, :])
        # stack head pairs into 128-partition tiles so Wo runs at K=128
        ctx2 = persist.tile([128, 4, S], BF16, tag="ctx2")
        for p in range(4):
            nc.sync.dma_start(out=ctx2[0:64, p, :], in_=ctxP[p][:, 0, :])
            nc.sync.dma_start(out=ctx2[64:128, p, :], in_=ctxP[p][:, 1, :])

        for lt in range(8):
            osb = bigp.tile([128, HID], BF16, tag="obig", name=f"osb{lt}")
            for nh2 in range(2):
                ps = psP.tile([128, 512], F32, tag="ps", name=f"ps_o_{lt}_{nh2}")
                for kc in range(4):
                    nc.tensor.matmul(
                        ps,
                        lhsT=ctx2[:, kc, 128 * lt:128 * lt + 128],
                        rhs=wo_sb[:, kc, 512 * nh2:512 * nh2 + 512],
                        start=(kc == 0), stop=(kc == 3),
                    )
                nc.scalar.copy(out=osb[:, 512 * nh2:512 * nh2 + 512], in_=ps)
            nc.sync.dma_start(out=attn_part[128 * lt:128 * lt + 128, :], in_=osb)
            if lt % 2 == 1:
                # quarter-granularity ReduceScatter: each collective starts as
                # soon as its two dense blocks land, pipelining CC stream
                # time with the remaining dense work and the LayerNorm
                q = lt // 2
                nc.gpsimd.collective_compute(
                    "ReduceScatter",
                    ALU.add,
                    replica_groups=[[0, 1], [2, 3], [4, 5], [6, 7]],
                    ins=[attn_part[256 * q:256 * q + 256, :]],
                    outs=[rs_out[128 * q:128 * q + 128, :]],
                )

        for lt in range(4):
            hsb = bigp.tile([128, HID], BF16, tag="big", name=f"hsb{lt}")
            nc.sync.dma_start(out=hsb, in_=rs_out[128 * lt:128 * lt + 128, :])
            rsb = bigp.tile([128, HID], BF16, tag="big", name=f"rsb{lt}")
            nc.sync.dma_start(out=rsb, in_=res[128 * lt:128 * lt + 128, :])
            h2 = bigp.tile([128, HID], F32, tag="obig", name=f"h2_{lt}")
            nc.vector.tensor_tensor(out=h2, in0=hsb, in1=rsb, op=ALU.add)

            stat = lns.tile([128, 16], F32, tag="stat", name=f"stat{lt}")
            for c in range(2):
                nc.vector.bn_stats(out=stat[:, 6 * c:6 * c + 6],
                                   in_=h2[:, 512 * c:512 * c + 512])
            mv = lns.tile([128, 4], F32, tag="mv", name=f"mv{lt}")
            nc.vector.bn_aggr(out=mv[:, 0:2],
                              in_=stat[:, 0:12].rearrange("p (n s) -> p n s", n=2))
            nc.scalar.activation(out=mv[:, 2:3], in_=mv[:, 1:2],
                                 func=AF.Sqrt, bias=eps_sb, scale=1.0)
            nc.vector.reciprocal(out=mv[:, 3:4], in_=mv[:, 2:3])

            xn = bigp.tile([128, HID], F32, tag="obig", name=f"xn{lt}")
            nc.vector.tensor_scalar(
                out=xn, in0=h2,
                scalar1=mv[:, 0:1], scalar2=mv[:, 3:4],
                op0=ALU.subtract, op1=ALU.mult,
            )
            xg = bigp.tile([128, HID], F32, tag="obig", name=f"xg{lt}")
            nc.vector.tensor_tensor(out=xg, in0=xn, in1=lng_bc, op=ALU.mult)
            ob = bigp.tile([128, HID], F32, tag="obig", name=f"ob{lt}")
            nc.vector.tensor_tensor(out=ob, in0=xg, in1=lnb_bc, op=ALU.add)
            nc.sync.dma_start(out=out[128 * lt:128 * lt + 128, :], in_=ob)

    nc.compile()
    return nc


def make_in_maps(hidden_states, attention_mask, Wq, bq, Wk, bk, Wv, bv,
                 dist_emb, Wo, bo, ln_g, ln_b):
    bf16 = ml_dtypes.bfloat16
    hs = np.ascontiguousarray(hidden_states, dtype=np.float32)
    mask = np.ascontiguousarray(attention_mask, dtype=np.float32)
    Wq = np.asarray(Wq, np.float32); Wk = np.asarray(Wk, np.float32)
    Wv = np.asarray(Wv, np.float32); Wo = np.asarray(Wo, np.float32)
    bq = np.asarray(bq, np.float32); bk = np.asarray(bk, np.float32)
    bv = np.asarray(bv, np.float32); bo = np.asarray(bo, np.float32)
    D = np.asarray(dist_emb, np.float32)
    ln_g = np.asarray(ln_g, np.float32); ln_b = np.asarray(ln_b, np.float32)

    z1 = np.zeros((1, HD), np.float32)
    dT = np.tile(np.concatenate([D, z1], 0).T, (2, 1)).astype(bf16)
    drT = np.tile(np.concatenate([D[::-1], z1], 0).T, (2, 1)).astype(bf16)
    ident8 = np.eye(128, dtype=np.float32).astype(ml_dtypes.float8_e4m3)

    in_maps = []
    for core in range(NCORES):
        b, g = core // 2, core % 2
        sl = slice(512 * g, 512 * g + 512)
        wvT_aug = np.zeros((HID, 520), np.float32)
        bv_aug = np.zeros(520, np.float32)
        for h in range(8):
            cs = 512 * g + 64 * h
            wvT_aug[:, 65 * h:65 * h + 64] = Wv[cs:cs + 64].T
            bv_aug[65 * h:65 * h + 64] = bv[cs:cs + 64]
            bv_aug[65 * h + 64] = 1.0
        in_maps.append({
            "hsT": np.ascontiguousarray(hs[b].T).astype(bf16),
            "res": (np.ascontiguousarray(
                np.concatenate([hs[b, 256 * q + 128 * g:256 * q + 128 * g + 128]
                                for q in range(4)], 0) + bo[None, :])
                    ).astype(bf16),
            "wqT": np.ascontiguousarray(Wq[sl].T).astype(bf16),
            "wkT": np.ascontiguousarray(Wk[sl].T).astype(bf16),
            "wvT": wvT_aug.astype(bf16),
            "bq": np.ascontiguousarray(bq[sl].reshape(4, 128).T),
            "bk": np.ascontiguousarray(bk[sl].reshape(4, 128).T),
            "bvaug": bv_aug,
            "drT": drT,
            "dT": dT,
            "woT": np.ascontiguousarray(Wo[:, sl].T.astype(bf16)),
            "maskc": np.ascontiguousarray(mask[b, 0, 0].reshape(8, 128).T),
            "ident8": ident8,
            "ones64": np.ones((128, 64), np.float32).astype(bf16),
            "lng": ln_g,
            "lnb": ln_b,
        })
    return in_maps


def kernel(**inputs):
    global _COMPILED
    if _COMPILED is None:
        _COMPILED = build_program()
    nc = _COMPILED
    in_maps = make_in_maps(**inputs)
    result = run_bass_kernel_spmd(nc, in_maps, core_ids=list(range(NCORES)))
    out = np.zeros((B, S, HID), np.float32)
    for core in range(NCORES):
        b, g = core // 2, core % 2
        shard = result.results[core]["out"]
        for q in range(4):
            out[b, 256 * q + 128 * g:256 * q + 128 * g + 128] = \
                shard[128 * q:128 * q + 128]
    return out
